# revision 20
# baseline (speedup 1.0000x reference)
"""Trainium2 Bass kernel for nn_EmbeddingBlock (gnn_message_passing).

Math:
  xe = emb_table[x]                              [N,H]
  pb = silu(pair_basis @ W_pair + b_pair)        [E,H]
  out = silu(concat(xe[i], xe[j], pb) @ W_emb + b_emb)

Key algebraic fold: xe[i] @ W_emb[0:H] == (emb_table @ W_emb[0:H])[x[i]], so
with T1 = emb_table@W1, T2 = emb_table@W2 and G[c1*105+c2] = T1[c1]+T2[c2]+b_emb
(11025 x 128 fp16 table) the per-edge math is silu(pb @ W3 + G[cls]),
cls = x[i]*105+x[j].  The G[cls] gather is done on the host (fp16, tiny table)
and shipped per-edge, pre-transposed; everything else runs on device.

Device layout is "transposed" (H on partitions, edges on free dim):
  - pair_basis shipped pre-transposed [16, E]
  - pbT = silu(W_pair-stationary matmul + b_pair)   ACT reads PSUM
  - psum_h = W3-stationary matmul over pbT          TensorE
  - h = psum_h + GtermT (DVE add, fp16 input), out = silu(h) (ACT)
  - DMA out [128, E] transposed; host de-transposes per-core outputs.
"""

import numpy as np

N_NODES = 100000
N_EDGES = 1000000
VOCAB = 105
OUT_DIM = 16
HIDDEN = 128
N_CORES = 8
E_CORE = N_EDGES // N_CORES          # 125000
SUPER = 1024                         # edges per super-tile
T_SUPER = -(-E_CORE // SUPER)        # 62
E_PAD = T_SUPER * SUPER              # 126976
N_CLS = VOCAB * VOCAB                # 11025

PROFILE = False                      # set True (from test.py) to NTFF-profile
LAST_RESULT = None                   # BassKernelResults of the last run

_compiled = None


def _build_program(e_pad=E_PAD, debug=False, act="Silu"):
    import concourse.bass as bass
    import concourse.mybir as mybir
    import concourse.tile as tile
    from concourse import bacc
    from concourse.bass import ts

    f32 = mybir.dt.float32
    f16 = mybir.dt.float16

    t_super = e_pad // SUPER

    nc = bacc.Bacc(
        "TRN2", target_bir_lowering=False, debug=debug, num_devices=N_CORES
    )

    pbt_d = nc.dram_tensor("pbt", [OUT_DIM, e_pad], f32, kind="ExternalInput").ap()
    gt_d = nc.dram_tensor("gterm", [128, e_pad], f16, kind="ExternalInput").ap()
    wp_d = nc.dram_tensor("wpair", [OUT_DIM, HIDDEN], f32, kind="ExternalInput").ap()
    w3_d = nc.dram_tensor("w3", [HIDDEN, HIDDEN], f32, kind="ExternalInput").ap()
    bp_d = nc.dram_tensor("bpair", [HIDDEN, 1], f32, kind="ExternalInput").ap()
    out_d = nc.dram_tensor("outt", [128, e_pad], f32, kind="ExternalOutput").ap()

    SILU = getattr(mybir.ActivationFunctionType, act)

    with tile.TileContext(nc) as tc:
        with (
            tc.tile_pool(name="const", bufs=1) as constp,
            tc.tile_pool(name="io", bufs=4) as iop,
            tc.tile_pool(name="work", bufs=3) as workp,
            tc.tile_pool(name="ps", bufs=2, space=bass.MemorySpace.PSUM) as psump,
        ):
            wp_sb = constp.tile([OUT_DIM, HIDDEN], f32, tag="wp")
            nc.sync.dma_start(wp_sb[:], wp_d[:])
            w3_sb = constp.tile([HIDDEN, HIDDEN], f32, tag="w3")
            nc.sync.dma_start(w3_sb[:], w3_d[:])
            bp_sb = constp.tile([HIDDEN, 1], f32, tag="bp")
            nc.sync.dma_start(bp_sb[:], bp_d[:])

            prev = None  # (h_sb, t) pending final silu + store
            for t in range(t_super):
                pb_in = iop.tile([OUT_DIM, SUPER], f32, tag="pbin")
                nc.gpsimd.dma_start(pb_in[:], pbt_d[:, ts(t, SUPER)])
                gt = iop.tile([128, SUPER], f16, tag="gt")
                nc.sync.dma_start(gt[:, : SUPER // 2], gt_d[:, ts(2 * t, SUPER // 2)])
                nc.sync.dma_start(gt[:, SUPER // 2 :], gt_d[:, ts(2 * t + 1, SUPER // 2)])

                ps_pb = psump.tile([128, SUPER], f32, tag="pspb")
                for k2 in range(SUPER // 512):
                    nc.tensor.matmul(
                        ps_pb[:, ts(k2, 512)], wp_sb[:], pb_in[:, ts(k2, 512)]
                    )
                pbt_sb = workp.tile([128, SUPER], f32, tag="pbts")
                nc.scalar.activation(pbt_sb[:], ps_pb[:], SILU, bias=bp_sb[:])

                ps_h = psump.tile([128, SUPER], f32, tag="psh")
                for k2 in range(SUPER // 512):
                    nc.tensor.matmul(
                        ps_h[:, ts(k2, 512)], w3_sb[:], pbt_sb[:, ts(k2, 512)]
                    )

                h_sb = workp.tile([128, SUPER], f32, tag="hsb")
                nc.vector.tensor_add(h_sb[:], ps_h[:], gt[:])

                # Lag the final silu+store by one super-tile so ACT never
                # stalls on the W3-matmul -> add chain of the same tile.
                if prev is not None:
                    ph, pt = prev
                    o_sb = workp.tile([128, SUPER], f32, tag="osb")
                    nc.scalar.activation(o_sb[:], ph[:], SILU)
                    nc.sync.dma_start(out_d[:, ts(2 * pt, SUPER // 2)], o_sb[:, : SUPER // 2])
                    nc.sync.dma_start(out_d[:, ts(2 * pt + 1, SUPER // 2)], o_sb[:, SUPER // 2 :])
                prev = (h_sb, t)

            ph, pt = prev
            o_sb = workp.tile([128, SUPER], f32, tag="osb")
            nc.scalar.activation(o_sb[:], ph[:], SILU)
            nc.sync.dma_start(out_d[:, ts(2 * pt, SUPER // 2)], o_sb[:, : SUPER // 2])
            nc.sync.dma_start(out_d[:, ts(2 * pt + 1, SUPER // 2)], o_sb[:, SUPER // 2 :])

    nc.compile()
    return nc


def _get_compiled():
    global _compiled
    if _compiled is None:
        _compiled = _build_program()
    return _compiled


def kernel(x, pair_basis, i, j, emb_table, W_pair, b_pair, W_emb, b_emb):
    global LAST_RESULT
    from concourse import bass_utils

    x = np.asarray(x)
    i = np.asarray(i)
    j = np.asarray(j)
    pair_basis = np.asarray(pair_basis, dtype=np.float32)
    emb_table = np.asarray(emb_table, dtype=np.float32)
    W_pair = np.asarray(W_pair, dtype=np.float32)
    b_pair = np.asarray(b_pair, dtype=np.float32)
    W_emb = np.asarray(W_emb, dtype=np.float32)
    b_emb = np.asarray(b_emb, dtype=np.float32)

    # ---- host fold: tiny table algebra + per-edge class gather ----
    T1 = emb_table @ W_emb[:HIDDEN]            # [V, H]
    T2 = emb_table @ W_emb[HIDDEN : 2 * HIDDEN]
    W3 = np.ascontiguousarray(W_emb[2 * HIDDEN :])  # [H, H]
    G = (T1[:, None, :] + T2[None, :, :] + b_emb).reshape(N_CLS, HIDDEN)
    G16 = G.astype(np.float16)

    cls = x[i].astype(np.int32) * VOCAB + x[j].astype(np.int32)
    gterm = G16[cls]                           # [E, H] fp16

    nc = _get_compiled()

    in_maps = []
    for c in range(N_CORES):
        sl = slice(c * E_CORE, (c + 1) * E_CORE)
        pbt = np.zeros((OUT_DIM, E_PAD), np.float32)
        pbt[:, :E_CORE] = pair_basis[sl].T
        gtt = np.zeros((128, E_PAD), np.float16)
        gtt[:, :E_CORE] = gterm[sl].T
        in_maps.append(
            {
                "pbt": pbt,
                "gterm": gtt,
                "wpair": W_pair,
                "w3": W3,
                "bpair": np.ascontiguousarray(b_pair.reshape(HIDDEN, 1)),
            }
        )

    res = bass_utils.run_bass_kernel_spmd(
        nc, in_maps, core_ids=list(range(N_CORES)), trace=PROFILE
    )
    LAST_RESULT = res

    out = np.empty((N_EDGES, HIDDEN), np.float32)
    for c in range(N_CORES):
        out[c * E_CORE : (c + 1) * E_CORE] = res.results[c]["outt"][:, :E_CORE].T
    return out
